# revision 39
# baseline (speedup 1.0000x reference)
"""Causal multi-head attention (B=2, T=4096, C=768, H=12) on 8 Trainium2 cores.

Sharding: core c handles batch b=c//4 and heads 3*(c%4)..3*(c%4)+2. The host
pre-transposes x (xT [C,T] bf16) and pre-slices/casts all weights to bf16, so
the kernel needs no on-chip transposition of x and every matmul runs bf16
(1 cyc/row at any free size on the PE, vs f32r needing free>=256).

Heads-outer passes; per (head, 512-token query strip):
  scores [k,q] = Kh^T-block @ Qh into PSUM f32; exp on ACT (diagonal blocks
  merged into 2 strided-AP exps per strip) -> pT bf16; AV in out-[q,d]
  orientation: ps_o[128q, 65] accumulates pT-block^T @ (V|1) over k-blocks --
  65 free cols per matmul instead of 512, halving AV tensor-engine time, and
  fully-masked diagonal blocks drop out of the k-loop. The softmax denominator
  comes from the appended ones column; normalization is a [128,1] reciprocal +
  per-partition scalar multiply; att [q,d] is transposed back to [d,q] on the
  PE (4 transposes into one zero-region-shared PSUM bank).

After each head's strips, one bf16 AllToAll ([8,64,512], 15us fixed + 25ns/KB
in the cost model) redistributes that head's output so core j holds all heads
for tq strip j. Phase 3 (Wo) accumulates per-head partials into an SBUF f32
accumulator while the next head computes -- its lt loads carry a dummy
data-dependency "gate" on late-pass attention so the tile scheduler cannot
hoist them ahead of the collective (the scheduler ignores emission order and
would stall the in-order PE queue for ~12us otherwise). Only the last head's
a2a + 1/3 of phase 3 sit on the critical-path tail.

Work is balanced across passes to keep the bottleneck engine (ACT: ~216us of
exp; PE: ~207us) fed: pass h0 carries xT loads + Q01/K01/V-h0 projections
(emitted in chunks woven between attention groups), pass h1 carries the
head-2 QK projection + V-h1, pass h2 carries V-h2, each one strip ahead of
use. PSUM: scores 2x[128,1024] + o 2x[128,512] (shared with the att
transpose bank) + proj/phase3 2x[128,512] = 8 banks exactly.
"""
import numpy as np
from contextlib import ExitStack

import ml_dtypes
import concourse.bass as bass
import concourse.mybir as mybir
import concourse.tile as tile
from concourse import bacc
from concourse.bass_utils import run_bass_kernel_spmd
from concourse.masks import make_identity, make_upper_triangular

T = 4096
C = 768
H = 12
D = 64
HPC = 3            # heads per core
MPC = HPC * D      # 192 projected dims per core
NCORES = 8
NTB = T // 128     # 32 k blocks
NQB = T // 512     # 8 tq strips
CB = C // 128      # 6 contraction blocks
f32 = mybir.dt.float32
bf16 = mybir.dt.bfloat16
fp8 = mybir.dt.float8e4
DR = mybir.MatmulPerfMode.DoubleRow
EXP = mybir.ActivationFunctionType.Exp
BF = ml_dtypes.bfloat16

_CACHE = {}


def _build():
    nc = bacc.Bacc(None, target_bir_lowering=False, num_devices=NCORES)
    xt_in = nc.declare_dram_parameter("xt", [C, T], fp8, isOutput=False)
    wq01_in = nc.declare_dram_parameter("wq01", [C, 128], fp8, isOutput=False)
    wk01_in = nc.declare_dram_parameter("wk01", [C, 128], fp8, isOutput=False)
    wqk2_in = nc.declare_dram_parameter("wqk2", [C, 128], fp8, isOutput=False)
    wv_in = nc.declare_dram_parameter("wv", [C, MPC], fp8, isOutput=False)
    wo_in = nc.declare_dram_parameter("wo", [C, C], bf16, isOutput=False)
    bq01_in = nc.declare_dram_parameter("bq01", [128], f32, isOutput=False)
    bk01_in = nc.declare_dram_parameter("bk01", [128], f32, isOutput=False)
    bqk2_in = nc.declare_dram_parameter("bqk2", [128], f32, isOutput=False)
    bv_in = nc.declare_dram_parameter("bv", [MPC], f32, isOutput=False)
    bo_in = nc.declare_dram_parameter("bo", [C], f32, isOutput=False)
    out_d = nc.declare_dram_parameter("out", [2, 512, C], f32, isOutput=True)

    with tile.TileContext(nc) as tc, ExitStack() as ctx:
        singles = ctx.enter_context(tc.tile_pool(name="singles", bufs=1))
        dram = ctx.enter_context(tc.tile_pool(name="dram", bufs=1, space="DRAM"))

        # ---- static tiles -------------------------------------------------
        identity_f = singles.tile([128, 128], f32)
        upper = singles.tile([128, 128], bf16)     # upper-tri (c >= r)
        zu = singles.tile([128, 256], bf16)        # [zeros | upper-tri]
        with tc.tile_pool(name="stage", bufs=1) as stage:
            make_identity(nc, identity_f)
            upf = stage.tile([128, 256], f32)
            nc.gpsimd.memset(upf[:, 0:128], 0.0)
            make_upper_triangular(nc, upf[:, 128:256], val=1.0)
            nc.vector.tensor_copy(upper, upf[:, 128:256])
            nc.vector.tensor_copy(zu, upf)
            # pre-trigger the exp table load so its ~1.3us hides in the ramp
            warm = stage.tile([1, 2], f32)
            nc.vector.memset(warm, 0.0)
            nc.scalar.activation(warm[:, 1:2], warm[:, 0:1], EXP, scale=1.0)

        # ---- weights -> SBUF (gpsimd/SWDGE queue, cheap dispatch) ---------
        wq01_r = singles.tile([128, 3, 2, 128], fp8)
        wk01_r = singles.tile([128, 3, 2, 128], fp8)
        wqk2_r = singles.tile([128, 3, 2, 128], fp8)
        wv_r = singles.tile([128, 3, 2, MPC], fp8)
        wo_r = singles.tile([128, CB, C], bf16)
        nc.gpsimd.dma_start(out=wq01_r, in_=wq01_in.rearrange("(a b p) m -> p a b m", a=3, b=2))
        nc.gpsimd.dma_start(out=wk01_r, in_=wk01_in.rearrange("(a b p) m -> p a b m", a=3, b=2))
        nc.gpsimd.dma_start(out=wqk2_r, in_=wqk2_in.rearrange("(a b p) m -> p a b m", a=3, b=2))
        nc.gpsimd.dma_start(out=wv_r, in_=wv_in.rearrange("(a b p) m -> p a b m", a=3, b=2))

        # ---- biases -------------------------------------------------------
        bq01_c = singles.tile([128, 1], f32)
        bk01_c = singles.tile([128, 1], f32)
        bqk2_c = singles.tile([128, 1], f32)
        nc.gpsimd.dma_start(out=bq01_c, in_=bq01_in[0:128].unsqueeze(1))
        nc.gpsimd.dma_start(out=bk01_c, in_=bk01_in[0:128].unsqueeze(1))
        nc.gpsimd.dma_start(out=bqk2_c, in_=bqk2_in[0:128].unsqueeze(1))
        bv_b = singles.tile([128, MPC], f32)
        nc.gpsimd.dma_start(
            out=bv_b,
            in_=bass.AP(tensor=bv_in.ap().tensor, offset=0, ap=[[0, 128]] + bv_in.ap().ap),
        )
        bo_b = singles.tile([128, C], f32)
        nc.gpsimd.dma_start(
            out=bo_b,
            in_=bass.AP(tensor=bo_in.ap().tensor, offset=0, ap=[[0, 128]] + bo_in.ap().ap),
        )

        # ---- persistent activations --------------------------------------
        q01 = singles.tile([128, T], bf16)   # rows 0:64 h0 Q^T, 64:128 h1 Q^T
        k01 = singles.tile([128, T], bf16)
        qk2 = singles.tile([128, T], bf16)   # rows 0:64 h2 Q^T (64:128 unused)
        k2t = singles.tile([64, T], bf16)    # h2 K^T at base partition 0
        v1 = singles.tile([128, NTB, HPC, D + 1], bf16)
        ones_t = singles.tile([128, NTB, HPC], bf16)
        nc.vector.memset(ones_t, 1.0)
        nc.vector.tensor_copy(v1[:, :, :, D], ones_t)
        # per-strip P^T buffers (ping-pong by strip parity)
        pt_bufs = (
            singles.tile([128, NTB, 512], bf16, name="pt0"),
            singles.tile([128, NTB, 512], bf16, name="pt1"),
        )
        acc = singles.tile([128, 8, C], f32)  # phase-3 accumulator (bb*4+tb)

        a2a_in = tuple(
            dram.tile([NCORES, D, 512], bf16, name=f"a2a_in{h}") for h in range(HPC)
        )
        a2a_out = tuple(
            dram.tile([NCORES, D, 512], bf16, name=f"a2a_out{h}") for h in range(HPC)
        )

        with (
            tc.tile_pool(name="pm", bufs=1) as pm,
            tc.tile_pool(name="psm", bufs=1, space="PSUM") as psm,
        ):
            # ---------------- projections (strip it), split into chunks ----
            def proj_chunks(it):
                xt_t = pm.tile([128, CB, 512], fp8, tag="xt", bufs=2, name="xt_t")

                def c_load():
                    xtr = xt_in.rearrange("(cb p) t -> p cb t", p=128)
                    nsub = 2 if (it == 0 and phase == 0) else 1
                    for hf in range(nsub):
                        w = CB // nsub
                        nc.sync.dma_start(
                            out=xt_t[:, w * hf : w * (hf + 1), :],
                            in_=xtr[:, w * hf : w * (hf + 1), 512 * it : 512 * (it + 1)],
                        )

                def mk_qk(w_r, bias_c, dst, split):
                    def chunk():
                        ps = psm.tile([128, 512], f32, tag="p", bufs=2, name="psP")
                        for c2 in range(3):
                            nc.tensor.matmul(
                                ps, w_r[:, c2], xt_t[:, 2 * c2 : 2 * c2 + 2, :],
                                start=(c2 == 0), stop=(c2 == 2),
                                perf_mode=DR,
                            )
                        if not split:
                            nc.vector.tensor_scalar_add(
                                dst[:, 512 * it : 512 * (it + 1)], ps, bias_c
                            )
                        else:
                            # rows 0:64 = h2 Q^T -> qk2; rows 64:128 = h2 K^T,
                            # relocated to base partition 0 via DMA
                            nc.vector.tensor_scalar_add(
                                dst[0:64, 512 * it : 512 * (it + 1)],
                                ps[0:64, :], bias_c[0:64],
                            )
                            ktmp = pm.tile([128, 512], bf16, tag="kt", bufs=2, name="ktmp")
                            nc.vector.tensor_scalar_add(
                                ktmp[64:128, :], ps[64:128, :], bias_c[64:128]
                            )
                            nc.sync.dma_start(
                                out=k2t[:, 512 * it : 512 * (it + 1)],
                                in_=ktmp[64:128, :],
                            )
                    return chunk

                def mk_v(tb):
                    def chunk():
                        ps = psm.tile([128, 512], f32, tag="p", bufs=2, name="psV")
                        for c2 in range(3):
                            nc.tensor.matmul(
                                ps[:, 0:MPC],
                                xt_t[:, 2 * c2 : 2 * c2 + 2, 128 * tb : 128 * (tb + 1)],
                                wv_r[:, c2],
                                start=(c2 == 0), stop=(c2 == 2),
                                perf_mode=DR,
                            )
                        tk = 4 * it + tb
                        nc.vector.tensor_add(
                            v1[:, tk, :, 0:D],
                            ps[:, 0:MPC].rearrange("p (h d) -> p h d", h=HPC),
                            bv_b.rearrange("p (h d) -> p h d", h=HPC),
                        )
                    return chunk

                return [
                    c_load,
                    mk_qk(wq01_r, bq01_c, q01, False),
                    mk_qk(wk01_r, bk01_c, k01, False),
                    mk_qk(wqk2_r, bqk2_c, qk2, True),
                    mk_v(0), mk_v(1), mk_v(2), mk_v(3),
                ]

            # ---------------- attention for (head h, strip s) ---------------
            def do_attn(h, s, interleave=()):
                """interleave: iterable of chunk-callables woven between
                score/exp groups to fill the PE during this strip."""
                inter = list(interleave)
                qh = (q01[0:64], q01[64:128], qk2[0:64])[h]
                kh = (k01[0:64], k01[64:128], k2t[0:64])[h]
                pt = pt_bufs[s % 2]
                ptf = pt.rearrange("p kb t -> p (kb t)")
                qs = qh[:, 512 * s : 512 * (s + 1)]
                nk = 4 * s + 4

                for p in range(2 * s):  # full k-block pairs
                    ik0 = 2 * p
                    ps2 = psm.tile([128, 1024], f32, tag="s", bufs=2, name="ps2")
                    nc.tensor.matmul(
                        ps2[:, 0:512], kh[:, 128 * ik0 : 128 * (ik0 + 1)], qs,
                        start=True, stop=True,
                    )
                    nc.tensor.matmul(
                        ps2[:, 512:1024], kh[:, 128 * (ik0 + 1) : 128 * (ik0 + 2)], qs,
                        start=True, stop=True,
                    )
                    nc.scalar.activation(
                        pt[:, ik0 : ik0 + 2, :].rearrange("p a b -> p (a b)"),
                        ps2, EXP, scale=0.125,
                    )
                    if inter:
                        inter.pop(0)()
                # diagonal: 4 blocks in 2 psum tiles / 2 exps
                d0 = 4 * s
                psA = psm.tile([128, 1024], f32, tag="s", bufs=2, name="psA")
                nc.tensor.matmul(
                    psA[:, 0:512], kh[:, 128 * d0 : 128 * (d0 + 1)], qs,
                    start=True, stop=True,
                )
                nc.tensor.matmul(
                    psA[:, 512:1024],
                    kh[:, 128 * (d0 + 1) : 128 * (d0 + 2)], qs,
                    start=True, stop=True,
                )
                nc.scalar.activation(
                    bass.AP(
                        tensor=ptf.tensor, offset=ptf[:, 512 * d0 :].offset,
                        ap=[ptf.ap[0], [512, 2], [1, 512]],
                    ),
                    psA.rearrange("p (a b) -> p a b", a=2),
                    EXP, scale=0.125,
                )
                psB = psm.tile([128, 1024], f32, tag="s", bufs=2, name="psB")
                nc.tensor.matmul(
                    psB[:, 256:512],
                    kh[:, 128 * (d0 + 2) : 128 * (d0 + 3)],
                    qh[:, 512 * s + 256 : 512 * (s + 1)],
                    start=True, stop=True,
                )
                nc.tensor.matmul(
                    psB[:, 768:1024],
                    kh[:, 128 * (d0 + 3) : 128 * (d0 + 4)],
                    qh[:, 512 * s + 256 : 512 * (s + 1)],
                    start=True, stop=True,
                )
                nc.scalar.activation(
                    bass.AP(
                        tensor=ptf.tensor,
                        offset=ptf[:, 512 * (d0 + 2) + 256 :].offset,
                        ap=[ptf.ap[0], [512, 2], [1, 256]],
                    ),
                    bass.AP(
                        tensor=psB.tensor, offset=psB[:, 256:].offset,
                        ap=[psB.ap[0], [512, 2], [1, 256]],
                    ),
                    EXP, scale=0.125,
                )
                # causal masks on the diagonal blocks
                nc.vector.tensor_mul(
                    pt[:, d0, 0:128], pt[:, d0, 0:128], upper)
                nc.vector.tensor_mul(
                    pt[:, d0 + 1, 128:256], pt[:, d0 + 1, 128:256], upper)
                nc.vector.tensor_mul(
                    pt[:, d0 + 2, 256:384], pt[:, d0 + 2, 256:384], upper)
                nc.vector.tensor_mul(
                    pt[:, d0 + 3, 256:512], pt[:, d0 + 3, 256:512], zu)
                while inter:
                    inter.pop(0)()

                # AV (out [q, 65]) + normalize, per q block
                att_st = pm.tile([128, 4, D], f32, tag="att", bufs=2, name="att_st")
                for qb in range(4):
                    if inter:
                        inter.pop(0)()
                    o_t = psm.tile([128, 512], f32, tag="o", bufs=2, name="o_t")
                    nkq = 4 * s + qb + 1
                    for ik in range(nkq):
                        nc.tensor.matmul(
                            o_t[:, 0:65],
                            pt[:, ik, 128 * qb : 128 * (qb + 1)],
                            v1[:, ik, h, :],
                            start=(ik == 0), stop=(ik == nkq - 1),
                        )
                    rc = pm.tile([128, 1], f32, tag="rc", bufs=4, name="rc")
                    nc.vector.reciprocal(rc, o_t[:, 64:65])
                    nc.vector.tensor_scalar_mul(att_st[:, qb, :], o_t[:, 0:64], rc)
                # transpose att [q,d] -> [d,q] into one PSUM bank (f32,
                # shares the "o" pool slot rotation)
                tp = psm.tile([128, 512], f32, tag="o", bufs=2, name="tp")
                for qb in range(4):
                    nc.tensor.matmul(
                        tp[0:64, 128 * qb : 128 * (qb + 1)],
                        att_st[:, qb, :], identity_f,
                        is_transpose=True, start=(qb == 0), stop=(qb == 3),
                    )
                attT = pm.tile([64, 512], bf16, tag="attT", bufs=3, name="attT")
                nc.vector.tensor_copy(attT, tp[0:64, 0:512])
                nc.sync.dma_start(out=a2a_in[h][s, :, :], in_=attT)
                return attT

            # ---------------- phase 3 partial for head h --------------------
            def phase3_chunks(h, gate=None):
                flat = a2a_out[h].rearrange("s d t -> (s d) t")  # [512, 512]
                lts = {}

                def mk_load(bb, half):
                    def chunk():
                        lt = pm.tile([128, 512], bf16, tag="lt", bufs=4, name="lt")
                        if gate is not None:
                            # dummy write gated on late attention output: pins
                            # the lt DMA (WAW) behind the gate tile (RAW) so
                            # the scheduler cannot hoist it ahead of the a2a
                            nc.vector.tensor_copy(lt[0:1, 0:1], gate[0:1, 0, 0:1])
                        q = (nc.scalar if half else nc.sync) if h == 2 else nc.sync
                        q.dma_start(
                            out=lt[:, 0:512],
                            in_=flat[
                                256 * bb + 128 * half : 256 * bb + 128 * (half + 1), :
                            ],
                        )
                        lts[(bb, half)] = lt
                    return chunk

                def mk(bb, tb):
                    def chunk():
                        if h == 2:
                            out_t = pm.tile([128, C], f32, tag="ot", bufs=4, name="ot")
                            ps3 = psm.tile([128, 1024], f32, tag="s", bufs=2, name="ps3")
                            for half in range(2):
                                lt_sl = lts[(bb, half)][:, 128 * tb : 128 * (tb + 1)]
                                nc.tensor.matmul(
                                    ps3[:, 0:512], lt_sl,
                                    wo_r[:, 2 * h + half, 0:512],
                                    start=(half == 0), stop=(half == 1),
                                )
                                nc.tensor.matmul(
                                    ps3[:, 512:768], lt_sl,
                                    wo_r[:, 2 * h + half, 512:C],
                                    start=(half == 0), stop=(half == 1),
                                )
                            nc.vector.tensor_add(
                                out_t, ps3[:, 0:768], acc[:, 4 * bb + tb, :]
                            )
                            oq = nc.gpsimd if (bb * 4 + tb) % 2 else nc.sync
                            oq.dma_start(
                                out=out_d[bb, 128 * tb : 128 * (tb + 1), :], in_=out_t
                            )
                            return
                        for och in range(2):
                            ps3 = psm.tile([128, 512], f32, tag="p", bufs=2, name="ps3")
                            for half in range(2):
                                nc.tensor.matmul(
                                    ps3[:, 0:384],
                                    lts[(bb, half)][:, 128 * tb : 128 * (tb + 1)],
                                    wo_r[:, 2 * h + half, 384 * och : 384 * (och + 1)],
                                    start=(half == 0), stop=(half == 1),
                                )
                            a_sl = acc[:, 4 * bb + tb, 384 * och : 384 * (och + 1)]
                            if h == 0:
                                nc.vector.tensor_add(
                                    a_sl, ps3[:, 0:384],
                                    bo_b[:, 384 * och : 384 * (och + 1)],
                                )
                            else:
                                nc.vector.tensor_add(a_sl, ps3[:, 0:384], a_sl)
                    return chunk

                return (
                    [mk_load(bb, half) for bb in range(2) for half in range(2)]
                    + [mk(bb, tb) for bb in range(2) for tb in range(4)]
                )

            # ---------------- main schedule ---------------------------------
            nc.gpsimd.dma_start(out=wo_r, in_=wo_in.rearrange("(cb p) m -> p cb m", p=128))
            # warm the PE p-state during the initial DMAs: ~4us of dummy
            # transposes so projections start at full clock
            warm_ps = psm.tile([128, 512], f32, tag="o", bufs=2, name="warm_ps")
            for w in range(6):
                nc.tensor.matmul(
                    warm_ps[:, 128 * (w % 4) : 128 * (w % 4 + 1)],
                    identity_f, identity_f,
                    is_transpose=True, start=(w == 0), stop=(w == 5),
                )
            for ch in proj_chunks(0):
                ch()
            for s in range(NQB):
                do_attn(0, s, interleave=proj_chunks(s + 1) if s < 7 else ())
            nc.gpsimd.collective_compute(
                "AllToAll", mybir.AluOpType.bypass,
                replica_groups=[list(range(NCORES))],
                ins=[a2a_in[0][:]], outs=[a2a_out[0][:]],
            )
            for h in (1, 2):
                p3 = None
                for s in range(NQB):
                    inter = p3[4:12] if (s == 7 and p3) else ()
                    gate = do_attn(h, s, interleave=inter)
                    if s == 5:
                        p3 = phase3_chunks(h - 1, gate=gate)
                        for ch in p3[0:4]:
                            ch()
                nc.gpsimd.collective_compute(
                    "AllToAll", mybir.AluOpType.bypass,
                    replica_groups=[list(range(NCORES))],
                    ins=[a2a_in[h][:]], outs=[a2a_out[h][:]],
                )
            for ch in phase3_chunks(2):
                ch()

    nc.finalize()
    return nc


def kernel(x, Wq, bq, Wk, bk, Wv, bv, Wo, bo):
    if "nc" not in _CACHE:
        _CACHE["nc"] = _build()
    nc = _CACHE["nc"]

    x = np.asarray(x, dtype=np.float32)
    Wq = np.asarray(Wq, np.float32)
    Wk = np.asarray(Wk, np.float32)
    Wv = np.asarray(Wv, np.float32)
    Wo = np.asarray(Wo, np.float32)
    # permute Wo rows from global order (192g + 64h + d) to the gathered
    # head-major layout (256h + 64g + d) used by phase 3
    perm = np.empty(C, dtype=np.int64)
    for h in range(HPC):
        for g in range(4):
            perm[256 * h + 64 * g : 256 * h + 64 * g + 64] = np.arange(
                MPC * g + D * h, MPC * g + D * h + D
            )
    F8 = ml_dtypes.float8_e4m3
    wo_send = np.ascontiguousarray(Wo[perm, :].astype(BF))
    xts = [np.ascontiguousarray(x[b].T.astype(F8)) for b in range(2)]
    in_maps = []
    for c in range(NCORES):
        b, g = c // 4, c % 4
        sl = slice(MPC * g, MPC * (g + 1))
        wq_g, wk_g = Wq[:, sl], Wk[:, sl]
        bq_g, bk_g = np.asarray(bq, np.float32)[sl], np.asarray(bk, np.float32)[sl]
        in_maps.append({
            "xt": xts[b],
            "wq01": np.ascontiguousarray(wq_g[:, 0:128].astype(F8)),
            "wk01": np.ascontiguousarray(wk_g[:, 0:128].astype(F8)),
            "wqk2": np.ascontiguousarray(
                np.concatenate([wq_g[:, 128:MPC], wk_g[:, 128:MPC]], axis=1).astype(F8)
            ),
            "wv": np.ascontiguousarray(Wv[:, sl].astype(F8)),
            "wo": wo_send,
            "bq01": np.ascontiguousarray(bq_g[0:128]),
            "bk01": np.ascontiguousarray(bk_g[0:128]),
            "bqk2": np.ascontiguousarray(
                np.concatenate([bq_g[128:MPC], bk_g[128:MPC]])
            ),
            "bv": np.ascontiguousarray(np.asarray(bv, np.float32)[sl]),
            "bo": np.ascontiguousarray(np.asarray(bo, np.float32)),
        })

    res = run_bass_kernel_spmd(nc, in_maps, core_ids=list(range(NCORES)))
    out = np.empty((2, T, C), dtype=np.float32)
    for j in range(NCORES):
        r = res.results[j]["out"]
        out[0, 512 * j : 512 * (j + 1), :] = r[0]
        out[1, 512 * j : 512 * (j + 1), :] = r[1]
    return out
